# revision 18
# baseline (speedup 1.0000x reference)
"""DFSMN layer Trainium2 kernel (8-core SPMD, batch-parallel).

Math: per batch b,
  h = x @ W^T + b_lin                      [L, H]
  out_pre[t] = h[t] + mem[t] + fut[t]  ==  (M @ h)[t]
    with M [L, L] banded: identity + past taps (50) + future taps (5),
    taps are scalars per lag: wm = mem_w.sum(-1), wf = la_w.sum(-1).
  out = LayerNorm_H(out_pre) * gamma + beta

On device (per core = one batch):
  g = x @ W^T + b    (bf16 TensorE into fp32 PSUM; bias added during the
                      DVE PSUM->SBUF evacuation so the band matmul sees h)
  pre = M @ h        (band width 56 < 128, so out tiles are offset by -64:
                      each out tile's source window [t0-50, t0+133) spans
                      exactly TWO aligned g tiles -> 2 matmuls instead of 3)
  out = (pre - mean) * rsqrt(var + eps)
                     (DVE bn_stats/bn_aggr straight from PSUM; apply on
                      ScalarE activation with per-partition scale/bias APs)
"""
import numpy as np
import ml_dtypes

MEM, LA, EPS = 50, 5, 1e-5
B, L, D, H = 8, 2048, 1024, 2048
NCORES = 8
PT = 128              # time tile (partition dim)
TB = L // PT          # 16 aligned time tiles
DC = D // PT          # 8 contract chunks
HN = 512              # matmul moving free dim (one PSUM bank)
HC = H // HN          # 4 H chunks
NO = TB + 1           # 17 offset out tiles; out tile j covers rows
                      # [j*128-64, j*128+64+... ) clipped to [0, L)

# Band blocks for offset out tiles: out tile j (1..15) covers rows
# [j*128-64, j*128+64) and sources aligned g tiles {j-1, j}.  The two
# 64-row edges (rows [0,64) and [L-64,L)) share one merged tile "E":
# partitions 0..63 <- rows [L-64, L) (sources g[15]), partitions 64..127
# <- rows [0, 64) (sources g[0]).  Block k is the [src 128, out 128]
# transposed slice of M (zero outside the valid rows).
BLOCK_LIST2 = ([(0, TB - 1), (0, 0)] +       # merged edge tile (j-key 0)
               [(j, sj) for j in range(1, TB) for sj in (j - 1, j)])
NBLK2 = len(BLOCK_LIST2)  # 32
BLOCKS_BY_J = {j: [(k, sj) for k, (j2, sj) in enumerate(BLOCK_LIST2) if j2 == j]
               for j in range(TB)}

_cached = {}
last_exec_time_ns = None


def _band_matrix(wm, wf):
    """M [L, L] fp32: out_pre = M @ h."""
    M = np.zeros((L, L), np.float32)
    idx = np.arange(L)
    M[idx, idx] = 1.0
    for t in range(L):
        if t < MEM:
            M[t, :t] += wm[:t]
        else:
            M[t, t - MEM:t] += wm
        hi = min(t + LA, L - 1)
        if hi >= t + 1:
            M[t, t + 1:hi + 1] += wf[:hi - t]
    return M


def _build_nc(reps=1, loop_k=None):
    from concourse import bacc
    import concourse.mybir as mybir
    import concourse.tile as tile

    dt = mybir.dt.bfloat16
    f32 = mybir.dt.float32

    nc = bacc.Bacc(None, target_bir_lowering=False)
    # x shipped transposed and t-tile-major: [TB, D, PT] so tile i's lhsT
    # slices are one small contiguous region per (i, dc).
    xtT = nc.declare_dram_parameter("xtT", [TB, D, PT], dt, isOutput=False)
    wT = nc.declare_dram_parameter("wT", [D, H], dt, isOutput=False)
    mT = nc.declare_dram_parameter("mT", [PT, NBLK2, PT], dt, isOutput=False)
    bv = nc.declare_dram_parameter("bv", [PT, H], f32, isOutput=False)
    out = nc.declare_dram_parameter("out", [L, H], dt, isOutput=True)

    with tile.TileContext(nc) as tc:
        with tc.tile_pool(name="const", bufs=1) as const, \
             tc.tile_pool(name="gpool", bufs=3) as gpool, \
             tc.tile_pool(name="opool", bufs=3) as opool, \
             tc.tile_pool(name="ln", bufs=2) as ln, \
             tc.tile_pool(name="psg", bufs=2, space="PSUM") as psg, \
             tc.tile_pool(name="psp", bufs=2, space="PSUM") as psp:

            wt_tiles = []
            for dc in range(DC):
                w = const.tile([PT, H], dt, tag=f"wt{dc}")
                nc.sync.dma_start(out=w, in_=wT[dc * PT:(dc + 1) * PT, :])
                wt_tiles.append(w)
            # x: one [128, TB, DC, PT] tile; per-t-tile coalesced DMA (256KB)
            # in t-tile-major order so tile 0's weights are ready early.
            xt_t = const.tile([PT, TB, DC, PT], dt, tag="xt")
            for i in range(TB):
                nc.sync.dma_start(
                    out=xt_t[:, i, :, :],
                    in_=xtT[i].rearrange("(dc p) t -> p dc t", p=PT))
            mt_t = const.tile([PT, NBLK2, PT], dt, tag="mt")
            nc.sync.dma_start(out=mt_t, in_=mT[:, :, :])
            bb_t = const.tile([PT, H], f32, tag="bb")
            nc.sync.dma_start(out=bb_t, in_=bv[:, :])
            eps_t = const.tile([PT, 1], f32, tag="eps")
            nc.vector.memset(eps_t, EPS)

            if loop_k is not None:
                # Unroll bodies inside the hw loop: For_i has an all-engine
                # barrier per iteration (~20us drain/refill); amortize it.
                unroll = 1
                for cand in (16, 8, 4, 3, 2):
                    if loop_k % cand == 0:
                        unroll = cand
                        break
                with tc.For_i(0, loop_k // unroll, 1):
                    for _u in range(unroll):
                        _emit_body(nc, mybir, xt_t, wt_tiles, mt_t, bb_t,
                                   eps_t, gpool, opool, ln, psg, psp, out)
            else:
                for _rep in range(reps):
                    _emit_body(nc, mybir, xt_t, wt_tiles, mt_t, bb_t,
                               eps_t, gpool, opool, ln, psg, psp, out)
    nc.finalize()
    return nc


def _emit_body(nc, mybir, xt_t, wt_tiles, mt_t, bb_t, eps_t,
               gpool, opool, ln, psg, psp, out):
    dt = mybir.dt.bfloat16
    f32 = mybir.dt.float32
    mult = mybir.AluOpType.mult
    add = mybir.AluOpType.add

    g_tiles = [None] * TB

    HP = 2 * HN  # 1024: two h-chunks share one 2-bank PSUM tile, halving
    NP = HC // 2  # the number of PSUM allocations (and group-start waits)

    def emit_g(i):
        gch = []
        for pair in range(NP):
            pg = psg.tile([PT, HP], f32, tag="pg")
            # matmul moving free dim is ISA-limited to 512 (s3d3_mm_num_elements),
            # so each 2-bank tile is filled by two 512-wide groups
            for half in range(2):
                hc = 2 * pair + half
                sl = slice(half * HN, (half + 1) * HN)
                for dc in range(DC):
                    nc.tensor.matmul(
                        pg[:, sl],
                        xt_t[:, i, dc, :],
                        wt_tiles[dc][:, hc * HN:(hc + 1) * HN],
                        start=(dc == 0), stop=(dc == DC - 1))
            # g[0] lives until the merged edge tile at the end of the rep,
            # so it gets its own ring instead of the 3-deep g ring.
            g = gpool.tile([PT, HP], dt, tag=(f"gz{pair}" if i == 0 else f"g{pair}"))
            # h = g + b during PSUM evacuation (DVE); band then sees h,
            # which propagates the bias through the taps exactly.
            nc.vector.tensor_tensor(
                out=g, in0=pg, in1=bb_t[:, pair * HP:(pair + 1) * HP], op=add)
            gch.append(g)
        g_tiles[i] = gch

    def emit_band(j):
        blist = BLOCKS_BY_J[j]
        stats = ln.tile([PT, HC, 6], f32, tag="stats")
        pres = []
        for pair in range(NP):
            pre = psp.tile([PT, HP], f32, tag="pre")
            for half in range(2):
                hc = 2 * pair + half
                sl = slice(half * HN, (half + 1) * HN)
                for bi, (k, sj) in enumerate(blist):
                    nc.tensor.matmul(
                        pre[:, sl], mt_t[:, k, :],
                        g_tiles[sj][pair][:, sl],
                        start=(bi == 0), stop=(bi == len(blist) - 1))
                nc.vector.bn_stats(out=stats[:, hc, :], in_=pre[:, sl])
            pres.append(pre)
        mv = ln.tile([PT, 2], f32, tag="mv")
        nc.vector.bn_aggr(out=mv, in_=stats)
        rstd = ln.tile([PT, 1], f32, tag="rstd")
        nc.scalar.activation(
            out=rstd, in_=mv[:, 1:2],
            func=mybir.ActivationFunctionType.Sqrt,
            bias=eps_t, scale=1.0)
        nc.vector.reciprocal(out=rstd, in_=rstd)
        nmr = ln.tile([PT, 1], f32, tag="nmr")
        # nmr = -mu * rstd  (so apply is one fused scale+bias on ScalarE)
        nc.vector.scalar_tensor_tensor(
            out=nmr, in0=mv[:, 0:1], scalar=-1.0, in1=rstd,
            op0=mult, op1=mult)
        o = opool.tile([PT, H], dt, tag="o")
        for pair in range(NP):
            nc.scalar.activation(
                out=o[:, pair * HP:(pair + 1) * HP], in_=pres[pair],
                func=mybir.ActivationFunctionType.Identity,
                bias=nmr, scale=rstd)
        if j == 0:
            # merged edge tile: partitions 0..63 are rows [L-64, L),
            # partitions 64..127 are rows [0, 64)
            nc.sync.dma_start(out=out[L - PT // 2:L, :], in_=o[0:PT // 2, :])
            nc.gpsimd.dma_start(out=out[0:PT // 2, :], in_=o[PT // 2:PT, :])
        else:
            r0 = j * PT - PT // 2
            eng = nc.sync if (j % 2 == 0) else nc.gpsimd
            eng.dma_start(out=out[r0:r0 + PT, :], in_=o)

    emit_g(0)
    for i in range(1, TB):
        emit_g(i)
        emit_band(i)
    emit_band(0)


def _get_runner(reps=1):
    """Compile once; return (run_fn, in_names, out_names).

    run_fn takes a list of global (concatenated-over-cores) jax/np arrays in
    in_names order followed by zero output buffers, returns global outputs.
    Mirrors concourse.bass2jax.run_bass_via_pjrt's multi-core branch, but
    keeps the jitted callable so repeated invocations don't rebuild/retrace.
    """
    key = ("runner", reps)
    if key in _cached:
        return _cached[key]

    import jax
    from jax.experimental.shard_map import shard_map
    from jax.sharding import Mesh, PartitionSpec
    import concourse.mybir as mybir
    from concourse import bass2jax

    if isinstance(reps, tuple):  # ("loop", K): hardware For_i timing variant
        nc = _build_nc(loop_k=reps[1])
    else:
        nc = _build_nc(reps)
    bass2jax.install_neuronx_cc_hook()

    partition_name = nc.partition_id_tensor.name if nc.partition_id_tensor else None
    in_names, out_names, out_avals, zero_outs = [], [], [], []
    for alloc in nc.m.functions[0].allocations:
        if not isinstance(alloc, mybir.MemoryLocationSet):
            continue
        name = alloc.memorylocations[0].name
        if alloc.kind == "ExternalInput":
            if name != partition_name:
                in_names.append(name)
        elif alloc.kind == "ExternalOutput":
            out_names.append(name)
            shape = tuple(alloc.tensor_shape)
            dtype = mybir.dt.np(alloc.dtype)
            out_avals.append(jax.core.ShapedArray(shape, dtype))
            zero_outs.append(np.zeros(shape, dtype))
    n_params = len(in_names)
    all_names = in_names + out_names
    if partition_name is not None:
        all_names.append(partition_name)

    def _body(*args):
        operands = list(args)
        if partition_name is not None:
            operands.append(bass2jax.partition_id_tensor())
        outs = bass2jax._bass_exec_p.bind(
            *operands,
            out_avals=tuple(out_avals),
            in_names=tuple(all_names),
            out_names=tuple(out_names),
            lowering_input_output_aliases=(),
            sim_require_finite=True,
            sim_require_nnan=True,
            nc=nc,
        )
        return tuple(outs)

    devices = jax.devices()[:NCORES]
    assert len(devices) == NCORES, f"need {NCORES} devices, have {len(jax.devices())}"
    mesh = Mesh(np.asarray(devices), ("core",))
    n_outs = len(out_names)
    fn = jax.jit(shard_map(
        _body, mesh=mesh,
        in_specs=(PartitionSpec("core"),) * (n_params + n_outs),
        out_specs=(PartitionSpec("core"),) * n_outs,
        check_rep=False))

    _cached[key] = (fn, in_names, out_names, zero_outs, mesh)
    return _cached[key]


def _prepare_in_arrays(x, W_lin, b_lin, wm, wf):
    """Host prep: per-core inputs concatenated over the core axis (axis 0)."""
    bf16 = ml_dtypes.bfloat16
    M = _band_matrix(wm, wf)
    mt_host = np.zeros((PT, NBLK2, PT), np.float32)
    edge_rows = np.r_[np.arange(L - PT // 2, L), np.arange(0, PT // 2)]
    for k, (j, sj) in enumerate(BLOCK_LIST2):
        if j == 0:
            # merged edge tile; far-off-band entries of M are zero, so the
            # full 128-col slice is correct for both halves
            mt_host[:, k, :] = M[edge_rows, sj * PT:(sj + 1) * PT].T
        else:
            o0 = j * PT - PT // 2
            mt_host[:, k, :] = M[o0:o0 + PT, sj * PT:(sj + 1) * PT].T
    per_core = {
        "wT": np.ascontiguousarray(W_lin.T).astype(bf16),
        "mT": mt_host.astype(bf16),
        "bv": np.broadcast_to(b_lin.reshape(1, H), (PT, H)).astype(np.float32),
    }
    arrays = {}
    # x: per-core transposed, t-tile-major: [B, TB, D, PT]
    xt = np.ascontiguousarray(
        x.reshape(B, TB, PT, D).transpose(0, 1, 3, 2)).astype(bf16)
    arrays["xtT"] = xt.reshape(B * TB, D, PT)
    for name, arr in per_core.items():
        arrays[name] = np.concatenate([arr] * NCORES, axis=0)
    return arrays


def _run(arrays):
    fn, in_names, out_names, zero_outs, _ = _get_runner()
    global_zero = [np.concatenate([z] * NCORES, axis=0) for z in zero_outs]
    args = [arrays[n] for n in in_names] + global_zero
    outs = fn(*args)
    return {n: np.asarray(o) for n, o in zip(out_names, outs)}


def kernel(x, W_lin, b_lin, mem_w, la_w, gamma, beta):
    x = np.asarray(x, np.float32)
    W_lin = np.asarray(W_lin, np.float32)
    b_lin = np.asarray(b_lin, np.float32)
    wm = np.asarray(mem_w, np.float32).sum(axis=-1, dtype=np.float32)
    wf = np.asarray(la_w, np.float32).sum(axis=-1, dtype=np.float32)
    gamma = np.asarray(gamma, np.float32)
    beta = np.asarray(beta, np.float32)

    arrays = _prepare_in_arrays(x, W_lin, b_lin, wm, wf)
    outs = _run(arrays)
    out = outs["out"].reshape(NCORES, L, H).astype(np.float32)

    # gamma/beta affine (trivial for the spec's ones/zeros fills; exact in general)
    if not np.all(gamma == 1.0):
        out = out * gamma[None, None, :]
    if not np.all(beta == 0.0):
        out = out + beta[None, None, :]
    return np.ascontiguousarray(out)
